# revision 1
# baseline (speedup 1.0000x reference)
"""BinaryConv2d (3x3, 64->64, SAME, binarized +-1 weights, bias+ReLU) on 8 TRN2 cores.

Strategy (data-parallel, 2 images per core):
  - Host: quantize W to +-1, cast x to fp16, build a row-pair-interleaved copy of
    the H-padded input so each on-chip row-pair buffer can be filled by a single
    2D-contiguous xbar DMA-transpose from DRAM.
  - Device per core: for each output row pair (2k, 2k+1), accumulate 6 fp16
    matmuls (K=128 = 2 stacked input rows x 64ch, M=128 = 2 output rows x 64ch,
    N=448 = 2 row-pairs x 224 px) into PSUM; ScalarE applies bias+ReLU and
    writes fp16; xbar DMA-transpose back to pixel-major; store NHWC fp16.
  - Host: widen fp16 -> fp32.

Layouts:
  xi  [2, 113, 224, 2, 64] fp16  xi[i, j, w, r, c] = xpad[i, 2j+r, w, c]
                                 (xpad has zero rows at 0 and 225)
  wm  [128, 768] fp16            six 128x128 lhsT blocks (a/dx0..2, b/dx0..2)
  bv  [128, 1] fp32              bias replicated to both row halves
  y   [2, 224, 224, 64] fp16
"""

import os

import numpy as np

import concourse.mybir as mybir
import concourse.tile as tile
from concourse import bacc
from concourse.bass_utils import run_bass_kernel_spmd

NCORES = 8
IMG = 2
H = W = 224
C = 64
NPB = 113          # row-pair buffers per image
WB = 256           # per-PB free stride in fp16 elems
COL0 = 16          # px 0 lives at col 16 (32B-aligned xbar dest); col 15/240 are zero pads
F16 = mybir.dt.float16
F32 = mybir.dt.float32

_cached = None


def _build():
    nc = bacc.Bacc("TRN2", target_bir_lowering=False, debug=False, num_devices=NCORES)
    xi = nc.dram_tensor("xi", [IMG, NPB, W, 2, C], F16, kind="ExternalInput")
    wm = nc.dram_tensor("wm", [128, 6 * 128], F16, kind="ExternalInput")
    bv = nc.dram_tensor("bv", [128, 1], F32, kind="ExternalInput")
    y = nc.dram_tensor("y", [IMG, H, W, C], F16, kind="ExternalOutput")

    with tile.TileContext(nc) as tc:
        with (
            tc.tile_pool(name="wp", bufs=1) as wp,
            tc.tile_pool(name="pbp", bufs=1) as pbp,
            tc.tile_pool(name="psp", bufs=8, space="PSUM") as psp,
            tc.tile_pool(name="sp", bufs=4) as sp,
            tc.tile_pool(name="tp", bufs=8) as tp,
        ):
            wt = wp.tile([128, 6 * 128], F16, tag="wt")
            nc.sync.dma_start(wt[:], wm[:])
            bt = wp.tile([128, 1], F32, tag="bt")
            nc.sync.dma_start(bt[:], bv[:])

            for img in range(IMG):
                pb = pbp.tile([128, NPB * WB], F16, tag=f"pb{img}")
                pbv = pb[:].rearrange("p (j w) -> p j w", w=WB)
                nc.gpsimd.memset(pbv[:, :, 15:16], 0)
                nc.gpsimd.memset(pbv[:, :, 240:241], 0)

                def emit_block(t):
                    ps = psp.tile([128, 448], F32, tag="ps")
                    for i in range(6):
                        ti = i            # 0..2 = type a (dx), 3..5 = type b (dx)
                        dx = i % 3
                        jbase = 2 * t if i < 3 else 2 * t + 1
                        rhs = pbv[:, jbase : jbase + 2, 15 + dx : 15 + dx + 224]
                        nc.tensor.matmul(
                            ps[:],
                            lhsT=wt[:, ti * 128 : (ti + 1) * 128],
                            rhs=rhs,
                            start=(i == 0),
                            stop=(i == 5),
                        )
                    s = sp.tile([128, 512], F16, tag="s")
                    sv = s[:].rearrange("p (q w) -> p q w", w=256)
                    nc.scalar.activation(
                        sv[:, :, 0:224],
                        ps[:].rearrange("p (q w) -> p q w", w=224),
                        mybir.ActivationFunctionType.Relu,
                        bias=bt[:],
                        scale=1.0,
                    )
                    for chunk in range(4):
                        pair = chunk // 2
                        half = chunk % 2
                        k = 2 * t + pair
                        npx = 128 if half == 0 else 96
                        to = tp.tile([128, 128], F16, tag="to")
                        nc.sync.dma_start(
                            to[:], s[:, chunk * 128 : (chunk + 1) * 128], transpose=True
                        )
                        dest = y[
                            img, 2 * k : 2 * k + 2, half * 128 : half * 128 + npx, :
                        ].rearrange("r w c -> w r c")
                        nc.scalar.dma_start(dest, to[0:npx, :])

                for j in range(NPB):
                    nc.sync.dma_start(
                        pb[:, j * WB + COL0 : j * WB + COL0 + 112],
                        xi[img, j, 0:112, :, :],
                        transpose=True,
                    )
                    nc.sync.dma_start(
                        pb[:, j * WB + COL0 + 112 : j * WB + COL0 + 224],
                        xi[img, j, 112:224, :, :],
                        transpose=True,
                    )
                    if j >= 2 and j % 2 == 0:
                        emit_block((j - 2) // 2)

    nc.compile()
    return nc


def _prep_inputs(x, Wf, b):
    x16 = x.astype(np.float16)
    n = x16.shape[0]
    xi = np.zeros((n, NPB, W, 2, C), dtype=np.float16)
    # xi[i, j, w, 0, c] = xpad row 2j   = orig row 2j-1  (j>=1)
    # xi[i, j, w, 1, c] = xpad row 2j+1 = orig row 2j    (j<=111)
    xi[:, 1:NPB, :, 0, :] = x16[:, 1:224:2, :, :]
    xi[:, 0 : NPB - 1, :, 1, :] = x16[:, 0:224:2, :, :]

    Wq = np.sign(Wf).astype(np.float16)  # [3(kh), 3(kw), 64(ci), 64(co)]
    wm = np.zeros((128, 6 * 128), dtype=np.float16)
    for dx in range(3):
        a = np.zeros((128, 128), dtype=np.float16)
        a[0:64, 0:64] = Wq[0, dx]
        a[64:128, 0:64] = Wq[1, dx]
        a[64:128, 64:128] = Wq[0, dx]
        wm[:, dx * 128 : (dx + 1) * 128] = a
        bb = np.zeros((128, 128), dtype=np.float16)
        bb[0:64, 0:64] = Wq[2, dx]
        bb[0:64, 64:128] = Wq[1, dx]
        bb[64:128, 64:128] = Wq[2, dx]
        wm[:, (3 + dx) * 128 : (4 + dx) * 128] = bb

    bv = np.concatenate([b, b]).astype(np.float32).reshape(128, 1)
    return xi, wm, bv


def kernel(x, W, b):
    global _cached
    if _cached is None:
        _cached = _build()
    nc = _cached

    xi, wm, bv = _prep_inputs(np.asarray(x), np.asarray(W), np.asarray(b))
    in_maps = [
        {"xi": np.ascontiguousarray(xi[IMG * core : IMG * (core + 1)]), "wm": wm, "bv": bv}
        for core in range(NCORES)
    ]
    trace = bool(os.environ.get("KERNEL_TRACE"))
    res = run_bass_kernel_spmd(
        nc, in_maps, core_ids=list(range(NCORES)), trace=trace
    )
    kernel.last_results = res
    out = np.concatenate([r["y"] for r in res.results], axis=0)
    return out.astype(np.float32)


# revision 7
# speedup vs baseline: 250.6999x; 250.6999x over previous
"""BinaryConv2d (3x3, 64->64, SAME, binarized +-1 weights, bias+ReLU) on 8 TRN2 cores.

Strategy (data-parallel, 2 images per core):
  - Host: quantize W to +-1 and pre-assemble six 128x128 lhsT blocks; cast x to
    fp16 and build a row-pair-interleaved, H- and W-padded copy so batched xbar
    DMA-transposes load channel-major row-pair buffers straight from DRAM.
  - Device per core, per image, 8 pipeline "slabs" (14 output row-pairs each):
      batched DRAM->SBUF xbar transposes fill 14 row-pair buffers (PB) at a time,
      7 blocks x 6 fp16 matmuls (K=128 = 2 input rows x 64ch, M=128 = 2 output
        rows x 64ch, N=448 = 2 row-pairs x 224px) accumulate in PSUM,
      ScalarE bias+ReLU+fp16-cast into a staging slab,
      1 batched SBUF->SBUF xbar transpose back to pixel-major,
      2 batched NHWC stores.
  - Host: widen fp16 -> fp32.

DRAM layouts:
  xi  [2, 114, 232, 2, 64] fp16   xi[i,j,1+w,r,c] = xpad[i, 2j+r, w, c]
                                  (xpad has zero rows at 0/225; w cols 0 and
                                   225..231 are zero; j=113 is all zero)
  wm  [128, 768] fp16             six 128x128 lhsT blocks (a/dx0..2, b/dx0..2)
  bv  [128, 1] fp32               bias replicated to both row halves
  y   [2, 224, 224, 64] fp16
"""

import os

import numpy as np

import concourse.mybir as mybir
import concourse.tile as tile
from concourse import bacc
from concourse.bass_utils import run_bass_kernel_spmd

NCORES = 8
IMG = 2
H = W = 224
C = 64
NPB = 114          # row-pair buffers per image (113 real + 1 zero pad)
WPX = 232          # per-PB width in px cols (1 zero + 224 data + 7 zero)
NPAIR = 112        # output row pairs per image
GS = 14            # pairs per output slab (8 slabs per image)
GI = 14            # PB buffers per input transpose chunk (8x14 + 1x2)
F16 = mybir.dt.float16
F32 = mybir.dt.float32

_cached = None


def _build(repeats=1):
    nc = bacc.Bacc("TRN2", target_bir_lowering=False, debug=False, num_devices=NCORES)
    xi = nc.dram_tensor("xi", [IMG, NPB, WPX, 2, C], F16, kind="ExternalInput")
    wm = nc.dram_tensor("wm", [128, 6 * 128], F16, kind="ExternalInput")
    bv = nc.dram_tensor("bv", [128, 1], F32, kind="ExternalInput")
    y = nc.dram_tensor("y", [IMG, H, W, C], F16, kind="ExternalOutput")

    with tile.TileContext(nc) as tc:
        with (
            tc.tile_pool(name="wp", bufs=1) as wp,
            tc.tile_pool(name="pbp", bufs=1) as pbp,
            tc.tile_pool(name="psp", bufs=8, space="PSUM") as psp,
            tc.tile_pool(name="sp", bufs=2) as sp,
            tc.tile_pool(name="tp", bufs=2) as tp,
        ):
            wt = wp.tile([128, 6 * 128], F16, tag="wt")
            nc.sync.dma_start(wt[:], wm[:])
            bt = wp.tile([128, 1], F32, tag="bt")
            nc.sync.dma_start(bt[:], bv[:])

            def in_chunk(img, pb, j0, j1):
                nc.sync.dma_start(
                    pb[:, j0 * WPX : j1 * WPX],
                    xi[img, j0:j1].rearrange("j w r c -> (j w) (r c)"),
                    transpose=True,
                )

            for _it in range(repeats * IMG):
                img = _it % IMG
                pb = pbp.tile([128, NPB * WPX], F16, tag=f"pb{img}")
                pbv = pb[:].rearrange("p (j w) -> p j w", w=WPX)

                in_chunk(img, pb, 0, GI)
                for s in range(8):
                    j0 = GI * (s + 1)
                    in_chunk(img, pb, j0, min(j0 + GI, NPB))

                    sb = sp.tile([128, GS * 256], F16, tag="sb")
                    sbv = sb[:].rearrange("p (g w) -> p g w", w=256)
                    for blk in range(7):
                        t = 7 * s + blk
                        ps = psp.tile([128, 448], F32, tag="ps")
                        for i in range(6):
                            dx = i % 3
                            jbase = 2 * t + (0 if i < 3 else 1)
                            rhs = pbv[:, jbase : jbase + 2, dx : dx + 224]
                            nc.tensor.matmul(
                                ps[:],
                                lhsT=wt[:, i * 128 : (i + 1) * 128],
                                rhs=rhs,
                                start=(i == 0),
                                stop=(i == 5),
                            )
                        nc.scalar.activation(
                            sbv[:, 2 * blk : 2 * blk + 2, 0:224],
                            ps[:].rearrange("p (q w) -> p q w", w=224),
                            mybir.ActivationFunctionType.Relu,
                            bias=bt[:],
                            scale=1.0,
                        )

                    # output transpose: sb [128, GS*256] -> st [128, 2*GS, 128]
                    st = tp.tile([128, 2 * GS * 128], F16, tag="st")
                    stv = st[:].rearrange("p (ch q) -> p ch q", q=128)
                    nc.sync.dma_start(stv[:, :, :], sb[:], transpose=True)

                    # 4 batched NHWC stores: pairs k in [14s, 14s+14), split by
                    # w-half (xbar chunk parity) and row parity (3-dim DMA APs)
                    k0 = GS * s
                    for rho in range(2):
                        rows = y[img, 2 * k0 + rho : 2 * (k0 + GS) : 2]
                        nc.scalar.dma_start(
                            rows[:, 0:128, :].rearrange("pl w c -> w pl c"),
                            stv[:, 0 : 2 * GS : 2, 64 * rho : 64 * rho + 64],
                        )
                        nc.scalar.dma_start(
                            rows[:, 128:224, :].rearrange("pl w c -> w pl c"),
                            stv[0:96, 1 : 2 * GS : 2, 64 * rho : 64 * rho + 64],
                        )

    nc.compile()
    return nc


def _prep_inputs(x, Wf, b):
    x16 = x.astype(np.float16)
    n = x16.shape[0]
    xi = np.zeros((n, NPB, WPX, 2, C), dtype=np.float16)
    # PB_j = padded rows (2j, 2j+1); padded row p = orig row p-1; px w at col w+1
    xi[:, 1:113, 1:225, 0, :] = x16[:, 1:224:2, :, :]
    xi[:, 0:112, 1:225, 1, :] = x16[:, 0:224:2, :, :]

    Wq = np.sign(Wf).astype(np.float16)  # [3(kh), 3(kw), 64(ci), 64(co)]
    wm = np.zeros((128, 6 * 128), dtype=np.float16)
    for dx in range(3):
        a = np.zeros((128, 128), dtype=np.float16)
        a[0:64, 0:64] = Wq[0, dx]
        a[64:128, 0:64] = Wq[1, dx]
        a[64:128, 64:128] = Wq[0, dx]
        wm[:, dx * 128 : (dx + 1) * 128] = a
        bb = np.zeros((128, 128), dtype=np.float16)
        bb[0:64, 0:64] = Wq[2, dx]
        bb[0:64, 64:128] = Wq[1, dx]
        bb[64:128, 64:128] = Wq[2, dx]
        wm[:, (3 + dx) * 128 : (4 + dx) * 128] = bb

    bv = np.concatenate([b, b]).astype(np.float32).reshape(128, 1)
    return xi, wm, bv


def kernel(x, W, b):
    global _cached
    if _cached is None:
        _cached = _build()
    nc = _cached

    xi, wm, bv = _prep_inputs(np.asarray(x), np.asarray(W), np.asarray(b))
    in_maps = [
        {"xi": np.ascontiguousarray(xi[IMG * core : IMG * (core + 1)]), "wm": wm, "bv": bv}
        for core in range(NCORES)
    ]
    trace = bool(os.environ.get("KERNEL_TRACE"))
    res = run_bass_kernel_spmd(nc, in_maps, core_ids=list(range(NCORES)), trace=trace)
    kernel.last_results = res
    out = np.concatenate([r["y"] for r in res.results], axis=0)
    return out.astype(np.float32)
